# revision 20
# baseline (speedup 1.0000x reference)
"""Trainium2 Bass kernel for a 4-term video/query contrastive loss.

Strategy (v5): data-parallel over batch B=64 across 8 cores (8 videos/core).
Device computes only the big cross-contrast term: 80 weight rows (64 queries
+ 16 local top-1 features) scored against all 8*2080 = 16640 upper-tri
proposal features, exp'd at 1/temperature, mask-reduced per
(video, {valid, iou>0.5}).

  - host pre-normalizes everything; V ships fp8e4 as [128, 2, 16640]
    (both C-halves interleaved -> each slab is ONE dma_start), W fp8
  - V is fully resident in SBUF (33 KB/partition); the load is split
    into 5 ranges across BOTH hardware DGE rings (sync + scalar) so
    per-DMA completion stalls overlap; compute starts on range 0
  - scores transposed S^T[p, r]: weights = fp8 v-chunks, stream = W
    (N=80), two C-half matmuls accumulate in PSUM; 12 chunks per
    2-bank PSUM group, one Exp ([128, 2, 480], scale=10) -> bf16
  - one mask matmul per 6-chunk subgroup: lhsT = [128, 96] block mask,
    rhs = et [128, 480], PSUM-accumulated into [96, 480]; host sums
    the 6 diagonal [16, 80] blocks
  - everything else (top-k, pos-pair cosines, small loss terms, logs)
    runs on the host in float64
"""

import numpy as np
import ml_dtypes

import concourse.bacc as bacc
import concourse.bass as bass
import concourse.tile as tile
from concourse import mybir
from concourse import bass_utils

f32 = mybir.dt.float32
bf16 = mybir.dt.bfloat16
AFT = mybir.ActivationFunctionType
BF = ml_dtypes.bfloat16

DT = mybir.dt.float8e4
DTNP = ml_dtypes.float8_e4m3

B, C, D = 64, 256, 64
T = 128
P = 2080                    # upper-tri positions per video
NCORES = 8
VB = B // NCORES            # videos per core: 8
NPT = 2                     # sentences per video
NPROP = VB * P              # 16640 proposals per core = 130 * 128 exactly
PCH = 128                   # proposals per chunk (partition dim of S^T)
NCH = NPROP // PCH          # 130 chunks
G = 6                       # chunks per mask subgroup (6*80 = 480 f32)
NG = (NCH + G - 1) // G     # 22 subgroups (21 full + 1 of 4)
NW = B + NPT * VB           # 80 score rows
MC = 2 * VB                 # 16 mask cols: (valid, iou>0.5) per local video
GM = G * MC                 # 96 block-mask cols per subgroup
TAU = 10.0
NEG_IOU = 0.5

SG = 2 * G                  # 12 chunks per super-group (2 PSUM banks)
NSG = (NCH + SG - 1) // SG  # 11 super-groups (10 full + 1 of 10)
BANKF = 512                 # f32 elements per PSUM bank

# v-load slabs (in chunks): (start, n, ring)  ring 0=sync, 1=scalar.
# Both hardware DGE rings together sustain ~410 GB/s, but they SHARE the 16
# SDMA engines, so ranges must alternate between rings in consumption order
# (each ring is FIFO). Scalar-ring issues beyond the first two are emitted
# inside the loop (between Exps) so they don't delay EXP(0).
VSLABS = [(0, 6, 0), (6, 12, 1), (18, 12, 0), (30, 12, 1), (42, 12, 0),
          (54, 12, 1), (66, 12, 0), (78, 12, 1), (90, 12, 0), (102, 12, 1),
          (114, 16, 0)]
N_SCALAR_UPFRONT = 1        # scalar v-slab issues emitted before the loop
MASK_LAG = 2                # super-groups of score-MM lead over mask MMs
NWARM = 8                   # N=512 dummy matmuls to lift the PE HAM clock gate


def _build_module():
    nc = bacc.Bacc("TRN2", target_bir_lowering=False, debug=False)

    d_v = nc.dram_tensor("v8", (PCH, 2, NPROP), DT, kind="ExternalInput")
    d_w = nc.dram_tensor("w8", (PCH, 2, NW), DT, kind="ExternalInput")
    d_m = nc.dram_tensor("msk", (PCH, NG * GM), bf16, kind="ExternalInput")
    d_or = nc.dram_tensor("o_r", (GM, G * NW), f32, kind="ExternalOutput")

    with tile.TileContext(nc) as tc:
        with (
            tc.tile_pool(name="consts", bufs=1) as cp,
            tc.tile_pool(name="ets", bufs=4) as ep,
            tc.tile_pool(name="outs", bufs=1) as op_,
            tc.tile_pool(name="ps", bufs=3, space="PSUM") as ps,
            tc.tile_pool(name="pr", bufs=1, space="PSUM") as pr,
            tc.tile_pool(name="pw", bufs=1, space="PSUM") as pw,
        ):
            wt = cp.tile([PCH, 2, NW], DT, tag="wt")
            nc.sync.dma_start(wt, d_w[:])
            mt = cp.tile([PCH, NG * GM], bf16, tag="mt")
            slab_of = {}
            vts = []
            deferred = []   # scalar-ring DMAs issued inside the loop
            n_scalar = 0
            for si, (c0, nch, ring) in enumerate(VSLABS):
                vtile = cp.tile([PCH, 2, nch * PCH], DT, tag=f"v{si}",
                                name=f"v{si}")
                a, b = c0 * PCH, (c0 + nch) * PCH
                if ring == 0:
                    nc.sync.dma_start(vtile, d_v[:, :, a:b])
                else:
                    n_scalar += 1
                    if n_scalar <= N_SCALAR_UPFRONT:
                        nc.scalar.dma_start(vtile, d_v[:, :, a:b])
                        if n_scalar == N_SCALAR_UPFRONT:
                            nc.scalar.dma_start(mt, d_m[:])
                    else:
                        deferred.append((vtile, a, b))
                vts.append((c0, vtile))
                for c in range(c0, c0 + nch):
                    slab_of[c] = si
            w0 = wt[:, 0, :]
            w1 = wt[:, 1, :]

            # Warm the PE clock gate (HAM) with ~3.7 us of dummy matmuls on a
            # memset scratch tile (no DMA dependency) while the first video
            # slabs stream in; the PE then starts the real work at 2.4 GHz.
            wsrc = cp.tile([128, 512], bf16, tag="wsrc")
            nc.vector.memset(wsrc, 0.0)
            warm = pw.tile([128, 512], f32, tag="warm")
            for i in range(NWARM):
                nc.tensor.matmul(warm, wsrc[:, 0:128], wsrc,
                                 start=True, stop=True)

            rsum = pr.tile([GM, G * NW], f32, tag="rs")
            ets = []  # et tile per super-group

            def mask_mm(g):
                sg, jj = divmod(g, 2)
                nc.tensor.matmul(rsum, mt[:, g * GM:(g + 1) * GM],
                                 ets[sg][:, jj, :],
                                 start=(g == 0), stop=(g == NG - 1))

            for sg in range(NSG):
                cg = min(SG, NCH - sg * SG)
                st = ps.tile([128, 2, BANKF], f32, tag="st")
                for j in range(cg):
                    c = sg * SG + j
                    jj, m = divmod(j, G)
                    c0, vtile = vts[slab_of[c]]
                    sl = slice((c - c0) * PCH, (c - c0 + 1) * PCH)
                    ds = st[:, jj, m * NW:(m + 1) * NW]
                    nc.tensor.matmul(ds, vtile[:, 0, sl], w0,
                                     start=(m == 0), stop=False)
                    nc.tensor.matmul(ds, vtile[:, 1, sl], w1,
                                     start=False, stop=(m == G - 1 or j == cg - 1))
                if sg < len(deferred):
                    vtile, a, b = deferred[sg]
                    nc.scalar.dma_start(vtile, d_v[:, :, a:b])
                et = ep.tile([128, 2, G * NW], bf16, tag="et")
                if cg < SG:
                    nc.vector.memset(et, 0.0)
                    nc.scalar.activation(et[:, 0, :], st[:, 0, 0:G * NW],
                                         AFT.Exp, scale=TAU)
                    r = cg - G
                    nc.scalar.activation(et[:, 1, 0:r * NW], st[:, 1, 0:r * NW],
                                         AFT.Exp, scale=TAU)
                else:
                    nc.scalar.activation(et[:, :, :], st[:, :, 0:G * NW],
                                         AFT.Exp, scale=TAU)
                ets.append(et)
                if sg >= MASK_LAG:
                    mask_mm(2 * (sg - MASK_LAG))
                    mask_mm(2 * (sg - MASK_LAG) + 1)
            for sg in range(NSG - MASK_LAG, NSG):
                mask_mm(2 * sg)
                mask_mm(2 * sg + 1)

            rs_sb = op_.tile([GM, G * NW], f32, tag="rsb")
            nc.vector.tensor_copy(rs_sb, rsum)
            nc.sync.dma_start(d_or[:], rs_sb)

    nc.compile()
    return nc


_MODULE = None


def _get_module():
    global _MODULE
    if _MODULE is None:
        _MODULE = _build_module()
    return _MODULE


def kernel(video_feats, query_feats, sents_feats, iou2d, iou2ds, num_targets):
    video_feats = np.ascontiguousarray(np.asarray(video_feats, np.float32))
    query_feats = np.asarray(query_feats, np.float32)
    sents_feats = np.asarray(sents_feats, np.float32)
    iou2d = np.asarray(iou2d, np.float32)
    iou2ds = np.asarray(iou2ds, np.float32)
    nt = np.asarray(num_targets)
    assert video_feats.shape == (B, C, D, D) and sents_feats.shape == (T, C)
    assert (nt == NPT).all(), "kernel assumes uniform num_targets == 2"

    rows, cols = np.triu_indices(D)
    tri = rows * D + cols

    vf = video_feats.reshape(B, C, D * D)[:, :, tri]           # (64, 256, 2080)
    nrm = np.sqrt(np.einsum('bcp,bcp->bp', vf, vf))
    vhat = vf / np.maximum(nrm, 1e-12)[:, None, :]

    qn = query_feats / np.maximum(
        np.linalg.norm(query_feats, axis=1, keepdims=True), 1e-12)
    sn = sents_feats / np.maximum(
        np.linalg.norm(sents_feats, axis=1, keepdims=True), 1e-12)

    iouf = iou2ds.reshape(T, D * D)[:, tri]
    pstar = iouf.argmax(1)                                     # top-1 per sentence
    scatter = np.repeat(np.arange(B), NPT)
    tvn = vhat[scatter, :, pstar]                              # (128, 256) normalized
    iou_tri = iou2d.reshape(B, D * D)[:, tri]
    posm_all = iou_tri > NEG_IOU

    vidx = np.repeat(np.arange(VB), P)
    ar = np.arange(NPROP)
    in_maps = []
    for k in range(NCORES):
        g0 = VB * k
        # (C, NPROP) -> [128, 2, NPROP] with dim1 = C-half
        vcat = vhat[g0:g0 + VB].transpose(1, 0, 2).reshape(2, PCH, NPROP)
        vcat = np.ascontiguousarray(vcat.transpose(1, 0, 2)).astype(DTNP)
        wk = np.concatenate([qn, tvn[2 * g0:2 * g0 + 2 * VB]], 0).T  # (256, 80)
        wk = np.ascontiguousarray(
            wk.reshape(2, PCH, NW).transpose(1, 0, 2)).astype(DTNP)  # (128,2,80)
        m = np.zeros((NG * G * PCH, MC), np.float32)
        pos = posm_all[g0:g0 + VB].reshape(-1)
        m[ar, 2 * vidx] = 1.0
        m[ar, 2 * vidx + 1] = pos
        mh = m.reshape(NG, G, PCH, MC).transpose(2, 0, 1, 3).reshape(
            PCH, NG * GM).astype(BF)
        in_maps.append({
            "v8": vcat,
            "w8": wk,
            "msk": np.ascontiguousarray(mh),
        })

    nc = _get_module()
    res = bass_utils.run_bass_kernel_spmd(nc, in_maps, core_ids=list(range(NCORES)))
    kernel._last = res
    outs = res.results

    # ---- host finalization (tiny, float64) ----
    E = np.float64
    valid = np.zeros((NCORES, VB, NW))
    posv = np.zeros((NCORES, VB, NW))
    for k in range(NCORES):
        rs = outs[k]["o_r"].astype(E)                          # (96, 480)
        acc = np.zeros((MC, NW))
        for j in range(G):
            acc += rs[MC * j:MC * (j + 1), NW * j:NW * (j + 1)]
        valid[k] = acc[0::2, :]
        posv[k] = acc[1::2, :]

    tvn64, qn64, sn64 = tvn.astype(E), qn.astype(E), sn.astype(E)
    negq = valid[:, :, :B].sum(axis=(0, 1))                    # (64,)
    for b in range(B):
        negq[b] -= posv[b // VB, b % VB, b]

    pos_t = (tvn64 * qn64[scatter]).sum(1)                     # (128,)
    E1 = np.exp(TAU * qn64 @ tvn64.T)                          # (64, 128)
    asum = E1.sum(0)
    t1 = -(TAU * pos_t - np.log(asum))
    t2 = -(TAU * pos_t - np.log(np.exp(TAU * pos_t) + negq[scatter]))

    a3 = tvn64 @ tvn64.T
    t3 = []
    for g in range(B):
        k, v = g // VB, g % VB
        for i in (NPT * g, NPT * g + 1):
            r = B + (i - 2 * VB * k)
            neg_i = valid[k, v, r] - posv[k, v, r]
            for j in (NPT * g, NPT * g + 1):
                pd = a3[i, j]
                t3.append(-(TAU * pd - np.log(np.exp(TAU * pd) + neg_i)))

    pos4 = (sn64 * qn64[scatter]).sum(1)
    E4 = np.exp(TAU * qn64 @ sn64.T)                           # (64, 128)
    mask4 = (scatter[None, :] != np.arange(B)[:, None])
    negsum4 = (E4 * mask4).sum(1)
    t4 = -(TAU * pos4 - np.log(np.exp(TAU * pos4) + negsum4[scatter]))

    return np.stack([t1.mean(), t2.mean(), np.mean(t3), t4.mean()]).astype(np.float32)


# revision 21
# speedup vs baseline: 1.1352x; 1.1352x over previous
"""Trainium2 Bass kernel for a 4-term video/query contrastive loss.

Strategy (v5): data-parallel over batch B=64 across 8 cores (8 videos/core).
Device computes only the big cross-contrast term: 80 weight rows (64 queries
+ 16 local top-1 features) scored against all 8*2080 = 16640 upper-tri
proposal features, exp'd at 1/temperature, mask-reduced per
(video, {valid, iou>0.5}).

  - host pre-normalizes everything; V ships fp8e4 as [128, 2, 16640]
    (both C-halves interleaved -> each slab is ONE dma_start), W fp8
  - V is fully resident in SBUF (33 KB/partition); the load is split
    into 5 ranges across BOTH hardware DGE rings (sync + scalar) so
    per-DMA completion stalls overlap; compute starts on range 0
  - scores transposed S^T[p, r]: weights = fp8 v-chunks, stream = W
    (N=80), two C-half matmuls accumulate in PSUM; 12 chunks per
    2-bank PSUM group, one Exp ([128, 2, 480], scale=10) -> bf16
  - one mask matmul per 6-chunk subgroup: lhsT = [128, 96] block mask,
    rhs = et [128, 480], PSUM-accumulated into [96, 480]; host sums
    the 6 diagonal [16, 80] blocks
  - everything else (top-k, pos-pair cosines, small loss terms, logs)
    runs on the host in float64
"""

import numpy as np
import ml_dtypes

import concourse.bacc as bacc
import concourse.bass as bass
import concourse.tile as tile
from concourse import mybir
from concourse import bass_utils

f32 = mybir.dt.float32
bf16 = mybir.dt.bfloat16
AFT = mybir.ActivationFunctionType
BF = ml_dtypes.bfloat16

DT = mybir.dt.float8e4
DTNP = ml_dtypes.float8_e4m3

B, C, D = 64, 256, 64
T = 128
P = 2080                    # upper-tri positions per video
NCORES = 8
VB = B // NCORES            # videos per core: 8
NPT = 2                     # sentences per video
NPROP = VB * P              # 16640 proposals per core = 130 * 128 exactly
PCH = 128                   # proposals per chunk (partition dim of S^T)
NCH = NPROP // PCH          # 130 chunks
G = 6                       # chunks per mask subgroup (6*80 = 480 f32)
NG = (NCH + G - 1) // G     # 22 subgroups (21 full + 1 of 4)
NW = B + NPT * VB           # 80 score rows
MC = 2 * VB                 # 16 mask cols: (valid, iou>0.5) per local video
GM = G * MC                 # 96 block-mask cols per subgroup
TAU = 10.0
NEG_IOU = 0.5

SG = 2 * G                  # 12 chunks per super-group (2 PSUM banks)
NSG = (NCH + SG - 1) // SG  # 11 super-groups (10 full + 1 of 10)
BANKF = 512                 # f32 elements per PSUM bank

# v-load slabs (in chunks): (start, n, ring)  ring 0=sync, 1=scalar.
# Both hardware DGE rings together sustain ~410 GB/s, but they SHARE the 16
# SDMA engines, so ranges must alternate between rings in consumption order
# (each ring is FIFO). Scalar-ring issues beyond the first two are emitted
# inside the loop (between Exps) so they don't delay EXP(0).
VSLABS = [(0, 4, 0), (4, 12, 1), (16, 12, 0), (28, 12, 1), (40, 12, 0),
          (52, 12, 1), (64, 12, 0), (76, 12, 1), (88, 12, 0), (100, 12, 1),
          (112, 18, 0)]
MASK_LAG = 2                # super-groups of score-MM lead over mask MMs


def _build_module():
    nc = bacc.Bacc("TRN2", target_bir_lowering=False, debug=False)

    d_v = nc.dram_tensor("v8", (PCH, NCH, 2, PCH), DT, kind="ExternalInput")
    d_w = nc.dram_tensor("w8", (PCH, 2, NW), DT, kind="ExternalInput")
    d_m = nc.dram_tensor("msk", (PCH, NG * GM), bf16, kind="ExternalInput")
    d_or = nc.dram_tensor("o_r", (GM, G * NW), f32, kind="ExternalOutput")

    with tile.TileContext(nc) as tc:
        with (
            tc.tile_pool(name="consts", bufs=1) as cp,
            tc.tile_pool(name="ets", bufs=4) as ep,
            tc.tile_pool(name="outs", bufs=1) as op_,
            tc.tile_pool(name="ps", bufs=3, space="PSUM") as ps,
            tc.tile_pool(name="pr", bufs=1, space="PSUM") as pr,
        ):
            wt = cp.tile([PCH, 2, NW], DT, tag="wt")
            nc.sync.dma_start(wt, d_w[:])
            mt = cp.tile([PCH, NG * GM], bf16, tag="mt")
            slab_of = {}
            vts = []
            n_scalar = 0
            for si, (c0, nch, ring) in enumerate(VSLABS):
                vtile = cp.tile([PCH, nch, 2, PCH], DT, tag=f"v{si}",
                                name=f"v{si}")
                if ring == 0:
                    nc.sync.dma_start(vtile, d_v[:, c0:c0 + nch])
                else:
                    n_scalar += 1
                    nc.scalar.dma_start(vtile, d_v[:, c0:c0 + nch])
                    if n_scalar == 1:
                        nc.scalar.dma_start(mt, d_m[:])
                vts.append((c0, vtile))
                for c in range(c0, c0 + nch):
                    slab_of[c] = si
            w0 = wt[:, 0, :]
            w1 = wt[:, 1, :]

            rsum = pr.tile([GM, G * NW], f32, tag="rs")
            ets = []  # et tile per super-group

            def mask_mm(g):
                sg, jj = divmod(g, 2)
                nc.tensor.matmul(rsum, mt[:, g * GM:(g + 1) * GM],
                                 ets[sg][:, jj, :],
                                 start=(g == 0), stop=(g == NG - 1))

            for sg in range(NSG):
                cg = min(SG, NCH - sg * SG)
                st = ps.tile([128, 2, BANKF], f32, tag="st")
                for j in range(cg):
                    c = sg * SG + j
                    jj, m = divmod(j, G)
                    c0, vtile = vts[slab_of[c]]
                    ds = st[:, jj, m * NW:(m + 1) * NW]
                    nc.tensor.matmul(ds, vtile[:, c - c0, 0, :], w0,
                                     start=(m == 0), stop=False)
                    nc.tensor.matmul(ds, vtile[:, c - c0, 1, :], w1,
                                     start=False, stop=(m == G - 1 or j == cg - 1))
                et = ep.tile([128, 2, G * NW], bf16, tag="et")
                if cg < SG:
                    nc.vector.memset(et, 0.0)
                    nc.scalar.activation(et[:, 0, :], st[:, 0, 0:G * NW],
                                         AFT.Exp, scale=TAU)
                    r = cg - G
                    nc.scalar.activation(et[:, 1, 0:r * NW], st[:, 1, 0:r * NW],
                                         AFT.Exp, scale=TAU)
                else:
                    nc.scalar.activation(et[:, :, :], st[:, :, 0:G * NW],
                                         AFT.Exp, scale=TAU)
                ets.append(et)
                if sg >= MASK_LAG:
                    mask_mm(2 * (sg - MASK_LAG))
                    mask_mm(2 * (sg - MASK_LAG) + 1)
            for sg in range(NSG - MASK_LAG, NSG):
                mask_mm(2 * sg)
                mask_mm(2 * sg + 1)

            rs_sb = op_.tile([GM, G * NW], f32, tag="rsb")
            nc.vector.tensor_copy(rs_sb, rsum)
            nc.sync.dma_start(d_or[:], rs_sb)

    nc.compile()
    return nc


_MODULE = None


def _get_module():
    global _MODULE
    if _MODULE is None:
        _MODULE = _build_module()
    return _MODULE


def kernel(video_feats, query_feats, sents_feats, iou2d, iou2ds, num_targets):
    video_feats = np.ascontiguousarray(np.asarray(video_feats, np.float32))
    query_feats = np.asarray(query_feats, np.float32)
    sents_feats = np.asarray(sents_feats, np.float32)
    iou2d = np.asarray(iou2d, np.float32)
    iou2ds = np.asarray(iou2ds, np.float32)
    nt = np.asarray(num_targets)
    assert video_feats.shape == (B, C, D, D) and sents_feats.shape == (T, C)
    assert (nt == NPT).all(), "kernel assumes uniform num_targets == 2"

    rows, cols = np.triu_indices(D)
    tri = rows * D + cols

    vf = video_feats.reshape(B, C, D * D)[:, :, tri]           # (64, 256, 2080)
    nrm = np.sqrt(np.einsum('bcp,bcp->bp', vf, vf))
    vhat = vf / np.maximum(nrm, 1e-12)[:, None, :]

    qn = query_feats / np.maximum(
        np.linalg.norm(query_feats, axis=1, keepdims=True), 1e-12)
    sn = sents_feats / np.maximum(
        np.linalg.norm(sents_feats, axis=1, keepdims=True), 1e-12)

    iouf = iou2ds.reshape(T, D * D)[:, tri]
    pstar = iouf.argmax(1)                                     # top-1 per sentence
    scatter = np.repeat(np.arange(B), NPT)
    tvn = vhat[scatter, :, pstar]                              # (128, 256) normalized
    iou_tri = iou2d.reshape(B, D * D)[:, tri]
    posm_all = iou_tri > NEG_IOU

    vidx = np.repeat(np.arange(VB), P)
    ar = np.arange(NPROP)
    in_maps = []
    for k in range(NCORES):
        g0 = VB * k
        # (C, NPROP) -> [128, NCH, 2, 128]: crow, chunk, C-half, proposal
        vcat = vhat[g0:g0 + VB].transpose(1, 0, 2).reshape(
            2, PCH, NCH, PCH).transpose(1, 2, 0, 3)
        vcat = np.ascontiguousarray(vcat).astype(DTNP)
        wk = np.concatenate([qn, tvn[2 * g0:2 * g0 + 2 * VB]], 0).T  # (256, 80)
        wk = np.ascontiguousarray(
            wk.reshape(2, PCH, NW).transpose(1, 0, 2)).astype(DTNP)  # (128,2,80)
        m = np.zeros((NG * G * PCH, MC), np.float32)
        pos = posm_all[g0:g0 + VB].reshape(-1)
        m[ar, 2 * vidx] = 1.0
        m[ar, 2 * vidx + 1] = pos
        mh = m.reshape(NG, G, PCH, MC).transpose(2, 0, 1, 3).reshape(
            PCH, NG * GM).astype(BF)
        in_maps.append({
            "v8": vcat,
            "w8": wk,
            "msk": np.ascontiguousarray(mh),
        })

    nc = _get_module()
    res = bass_utils.run_bass_kernel_spmd(nc, in_maps, core_ids=list(range(NCORES)))
    kernel._last = res
    outs = res.results

    # ---- host finalization (tiny, float64) ----
    E = np.float64
    valid = np.zeros((NCORES, VB, NW))
    posv = np.zeros((NCORES, VB, NW))
    for k in range(NCORES):
        rs = outs[k]["o_r"].astype(E)                          # (96, 480)
        acc = np.zeros((MC, NW))
        for j in range(G):
            acc += rs[MC * j:MC * (j + 1), NW * j:NW * (j + 1)]
        valid[k] = acc[0::2, :]
        posv[k] = acc[1::2, :]

    tvn64, qn64, sn64 = tvn.astype(E), qn.astype(E), sn.astype(E)
    negq = valid[:, :, :B].sum(axis=(0, 1))                    # (64,)
    for b in range(B):
        negq[b] -= posv[b // VB, b % VB, b]

    pos_t = (tvn64 * qn64[scatter]).sum(1)                     # (128,)
    E1 = np.exp(TAU * qn64 @ tvn64.T)                          # (64, 128)
    asum = E1.sum(0)
    t1 = -(TAU * pos_t - np.log(asum))
    t2 = -(TAU * pos_t - np.log(np.exp(TAU * pos_t) + negq[scatter]))

    a3 = tvn64 @ tvn64.T
    t3 = []
    for g in range(B):
        k, v = g // VB, g % VB
        for i in (NPT * g, NPT * g + 1):
            r = B + (i - 2 * VB * k)
            neg_i = valid[k, v, r] - posv[k, v, r]
            for j in (NPT * g, NPT * g + 1):
                pd = a3[i, j]
                t3.append(-(TAU * pd - np.log(np.exp(TAU * pd) + neg_i)))

    pos4 = (sn64 * qn64[scatter]).sum(1)
    E4 = np.exp(TAU * qn64 @ sn64.T)                           # (64, 128)
    mask4 = (scatter[None, :] != np.arange(B)[:, None])
    negsum4 = (E4 * mask4).sum(1)
    t4 = -(TAU * pos4 - np.log(np.exp(TAU * pos4) + negsum4[scatter]))

    return np.stack([t1.mean(), t2.mean(), np.mean(t3), t4.mean()]).astype(np.float32)
